# revision 4
# baseline (speedup 1.0000x reference)
"""GQA attention block (B=2, S=2048, H=2048, NH=16, NKV=4, HD=128) on 8 TRN2
NeuronCores.

Sharding: 2 batch groups x 4-way tensor parallel over heads.
Core c = b*4 + l handles batch b, q-heads [4l, 4l+4), kv-head l, and computes
output columns [512l, 512(l+1)) of y[b] after an AllGather of the per-head
context over its 4-core group.

Per-core device pipeline (all layouts chosen so no on-device transposes of x/W
are needed -- the host passes xT / W.T shards):
  phase 1: qT/kT/vT = W.T-shards^T @ xT   (f32r matmuls, PSUM accum over h)
           epilogue adds position bias to qT,kT (DVE, writes f32r)
           vT chunks PE-transposed to v[j,d] bf16
  phase 2: per head: S = qT.T @ kT (f32r), row-max (DVE), exp+row-sum (ACT,
           bias/scale fused), A normalized (DVE), A.T via PE transpose (bf16),
           ctx.T = v.T-chunks @ A.T (bf16), ctx -> DRAM bounce (bf16)
  comm:    per-head AllGather of ctx.T across the 4-core group (overlaps)
  phase 3: y-slice = ctx_full.T @ Wo-slice.T (bf16), streamed from gathered
           bounce buffers.

Position-bias numerics: k-side bias is centered (c'_j = 0.01*pos_j - mid) --
the dropped term is constant along the softmax axis, and centering halves the
f32r rounding noise of the huge (pos_i*pos_j*HD) score component.
"""
import os
import numpy as np

import concourse.bass as bass
import concourse.mybir as mybir
from concourse import bacc, tile
from concourse.bass_utils import run_bass_kernel_spmd

import ml_dtypes

F32 = mybir.dt.float32
F32R = mybir.dt.float32r
BF16 = mybir.dt.bfloat16
AF = mybir.ActivationFunctionType

B, S, H = 2, 2048, 2048
NH, NKV, HD = 16, 4, 128
TP = 4                      # tensor-parallel group size
QH = NH // TP               # q heads per core (4)
OSL = H // TP               # output cols per core (512)
SCALE = 1.0 / np.sqrt(HD)
NHC = H // 128              # 16 contraction chunks of 128
NIT = S // 128              # 16 i-tiles
NJS = S // 512              # 4 j-slices of 512
NISL = S // 512             # 4 i-slices of 512

_CACHED = {}


def _build(mask_mode):
    """mask_mode: 'ones' (ignore mask) or 'binary' (additive -1e9 bias)."""
    nc = bacc.Bacc("TRN2", target_bir_lowering=False, debug=False, num_devices=8)

    xt = nc.dram_tensor("xt", [H, S], F32R, kind="ExternalInput")
    wqt = nc.dram_tensor("wqt", [H, OSL], F32R, kind="ExternalInput")
    wkt = nc.dram_tensor("wkt", [H, HD], F32R, kind="ExternalInput")
    wvt = nc.dram_tensor("wvt", [H, HD], F32R, kind="ExternalInput")
    wot = nc.dram_tensor("wot", [H, OSL], F32, kind="ExternalInput")
    posq = nc.dram_tensor("posq", [128, S], F32, kind="ExternalInput")
    posk = nc.dram_tensor("posk", [128, S], F32, kind="ExternalInput")
    maskb = nc.dram_tensor("maskb", [1, S], F32, kind="ExternalInput")
    out = nc.dram_tensor("out", [S, OSL], F32, kind="ExternalOutput")

    idb_dram = nc.inline_tensor(np.eye(128, dtype=ml_dtypes.bfloat16), name="idb")

    groups = [[0, 1, 2, 3], [4, 5, 6, 7]]

    with tile.TileContext(nc) as tc:
        with (
            tc.tile_pool(name="pers", bufs=1) as pers,
            tc.tile_pool(name="small", bufs=12) as small,
            tc.tile_pool(name="dram", bufs=1, space="DRAM") as dram,
        ):
            # ---------------- persistent tiles ----------------
            qt_sb = pers.tile([128, QH, S], F32R)       # [d, h, i]  4MB
            kt_sb = pers.tile([128, S], F32R)           # [d, j]     1MB
            v_sb = pers.tile([128, NHC, HD], BF16)      # [j, jc, d] 0.5MB
            idb_sb = pers.tile([128, 128], BF16)
            nc.sync.dma_start(idb_sb[:], idb_dram.ap())

            # AG bounce buffers (per head)
            cin = [dram.tile([128, S], BF16, name=f"cin{h}") for h in range(QH)]
            gout = [dram.tile([TP * 128, S], BF16, name=f"gout{h}") for h in range(QH)]

            # ---------------- phase 1: QKV projections ----------------
            with (
                tc.tile_pool(name="p1w", bufs=1) as p1w,
                tc.tile_pool(name="p1x", bufs=3) as p1x,
                tc.tile_pool(name="p1ps", bufs=1, space="PSUM") as p1ps,
                tc.tile_pool(name="p1st", bufs=2) as p1st,
            ):
                wq_sb = p1w.tile([128, NHC, OSL], F32R)
                wk_sb = p1w.tile([128, NHC, HD], F32R)
                wv_sb = p1w.tile([128, NHC, HD], F32R)
                posq_sb = p1w.tile([128, S], F32)
                posk_sb = p1w.tile([128, S], F32)
                nc.sync.dma_start(
                    wq_sb[:], wqt[:].rearrange("(c p) o -> p c o", p=128))
                nc.sync.dma_start(
                    wk_sb[:], wkt[:].rearrange("(c p) o -> p c o", p=128))
                nc.sync.dma_start(
                    wv_sb[:], wvt[:].rearrange("(c p) o -> p c o", p=128))
                nc.sync.dma_start(posq_sb[:], posq[:])
                nc.sync.dma_start(posk_sb[:], posk[:])

                vt_stage = p1w.tile([128, S], BF16)      # vT [d, j] staged

                for isl in range(4):
                    i0 = isl * 512
                    xt_tiles = []
                    qp = [p1ps.tile([128, 512], F32, tag=f"q{o}", name=f"qp{o}") for o in range(QH)]
                    kp = p1ps.tile([128, 512], F32, tag="k")
                    vp = p1ps.tile([128, 512], F32, tag="v")
                    for hc in range(NHC):
                        xt_t = p1x.tile([128, 512], F32R)
                        nc.sync.dma_start(
                            xt_t[:], xt[hc * 128:(hc + 1) * 128, i0:i0 + 512])
                        st = hc == 0
                        sp = hc == NHC - 1
                        for o in range(QH):
                            nc.tensor.matmul(
                                qp[o][:], wq_sb[:, hc, o * 128:(o + 1) * 128],
                                xt_t[:], start=st, stop=sp)
                        nc.tensor.matmul(kp[:], wk_sb[:, hc, :], xt_t[:],
                                         start=st, stop=sp)
                        nc.tensor.matmul(vp[:], wv_sb[:, hc, :], xt_t[:],
                                         start=st, stop=sp)
                    # epilogues: pos-add (DVE, f32r out); v staged bf16 (ACT)
                    for o in range(QH):
                        nc.vector.scalar_tensor_tensor(
                            qt_sb[:, o, i0:i0 + 512], qp[o][:], 1.0,
                            posq_sb[:, i0:i0 + 512],
                            op0=mybir.AluOpType.mult, op1=mybir.AluOpType.add)
                    nc.vector.scalar_tensor_tensor(
                        kt_sb[:, i0:i0 + 512], kp[:], 1.0,
                        posk_sb[:, i0:i0 + 512],
                        op0=mybir.AluOpType.mult, op1=mybir.AluOpType.add)
                    nc.scalar.copy(vt_stage[:, i0:i0 + 512], vp[:])

                # v transpose: vT [d, j] -> v [j, d] per 128-chunk
                with tc.tile_pool(name="p1tp", bufs=2, space="PSUM") as p1tp:
                    for jc in range(NHC):
                        tp = p1tp.tile([128, 128], BF16)
                        nc.tensor.transpose(
                            tp[:], vt_stage[:, jc * 128:(jc + 1) * 128], idb_sb[:])
                        nc.vector.tensor_copy(v_sb[:, jc, :], tp[:])

            # mask bias (binary mode): additive row [1, S] f32r for accum-MM
            if mask_mode == "binary":
                maskb_sb = pers.tile([1, S], F32R)
                ones1_sb = pers.tile([1, 128], F32R)
                nc.gpsimd.dma_start(maskb_sb[:], maskb[:])
                ones_dram = nc.inline_tensor(
                    np.ones((1, 128), dtype=np.float32), name="ones1")
                nc.gpsimd.dma_start(ones1_sb[:], ones_dram.ap())

            # ---------------- phase 2: attention per head ----------------
            # preload wot (phase 3 weights) early; cast to bf16
            wo_sb = pers.tile([128, NHC, OSL], BF16)
            nc.gpsimd.dma_start(
                wo_sb[:], wot[:].rearrange("(c p) o -> p c o", p=128))

            with (
                tc.tile_pool(name="p2s", bufs=1, space="PSUM") as p2s,
                tc.tile_pool(name="p2tp", bufs=2, space="PSUM") as p2tp,
                tc.tile_pool(name="p2cx", bufs=2, space="PSUM") as p2cx,
                tc.tile_pool(name="p2a", bufs=5) as p2a,
                tc.tile_pool(name="p2at", bufs=2) as p2at,
                tc.tile_pool(name="p2ctx", bufs=2) as p2ctx,
            ):
                for h in range(QH):
                    ctxT = p2ctx.tile([128, S], BF16)
                    for isl in range(NISL):
                        a_tiles = []
                        for it4 in range(4):
                            it = isl * 4 + it4
                            Sp = p2s.tile([128, S], F32)
                            for js in range(NJS):
                                nc.tensor.matmul(
                                    Sp[:, js * 512:(js + 1) * 512],
                                    qt_sb[:, h, it * 128:(it + 1) * 128],
                                    kt_sb[:, js * 512:(js + 1) * 512],
                                    start=True,
                                    stop=(mask_mode != "binary"))
                                if mask_mode == "binary":
                                    nc.tensor.matmul(
                                        Sp[:, js * 512:(js + 1) * 512],
                                        ones1_sb[:],
                                        maskb_sb[:, js * 512:(js + 1) * 512],
                                        start=False, stop=True,
                                        skip_group_check=True)
                            m4 = small.tile([128, 4], F32)
                            for js in range(NJS):
                                nc.vector.reduce_max(
                                    m4[:, js:js + 1],
                                    Sp[:, js * 512:(js + 1) * 512],
                                    axis=mybir.AxisListType.X)
                            m1 = small.tile([128, 1], F32)
                            nc.vector.reduce_max(
                                m1[:], m4[:], axis=mybir.AxisListType.X)
                            negm_s = small.tile([128, 1], F32)
                            nc.vector.tensor_scalar_mul(negm_s[:], m1[:], -SCALE)

                            A = p2a.tile([128, S], BF16)
                            sums = small.tile([128, 1], F32)
                            nc.scalar.activation(
                                A[:], Sp[:], AF.Exp,
                                bias=negm_s[:], scale=SCALE, accum_out=sums[:])
                            r = small.tile([128, 1], F32)
                            nc.vector.reciprocal(r[:], sums[:])
                            nc.vector.tensor_scalar_mul(A[:], A[:], r[:])
                            a_tiles.append(A)

                        AT = p2at.tile([128, NHC, 512], BF16)
                        for jc in range(NHC):
                            tp = p2tp.tile([128, 512], BF16)
                            for it4 in range(4):
                                nc.tensor.transpose(
                                    tp[:, it4 * 128:(it4 + 1) * 128],
                                    a_tiles[it4][:, jc * 128:(jc + 1) * 128],
                                    idb_sb[:])
                            if jc % 2 == 0:
                                nc.vector.tensor_copy(AT[:, jc, :], tp[:])
                            else:
                                nc.scalar.copy(AT[:, jc, :], tp[:])

                        ctxp = p2cx.tile([128, 512], F32)
                        for jc in range(NHC):
                            nc.tensor.matmul(
                                ctxp[:], v_sb[:, jc, :], AT[:, jc, :],
                                start=(jc == 0), stop=(jc == NHC - 1))
                        nc.scalar.copy(
                            ctxT[:, isl * 512:(isl + 1) * 512], ctxp[:])

                    nc.sync.dma_start(cin[h][:], ctxT[:])
                    nc.gpsimd.collective_compute(
                        "AllGather", mybir.AluOpType.bypass,
                        ins=[cin[h][:].opt()], outs=[gout[h][:].opt()],
                        replica_groups=groups)

            # ---------------- phase 3: output projection ----------------
            with (
                tc.tile_pool(name="p3c", bufs=6) as p3c,
                tc.tile_pool(name="p3y", bufs=3, space="PSUM") as p3y,
                tc.tile_pool(name="p3o", bufs=3) as p3o,
            ):
                for it in range(NIT):
                    yp = p3y.tile([128, OSL], F32)
                    for cc in range(NHC):
                        a, lr = cc // 4, cc % 4
                        cf = p3c.tile([128, 128], BF16)
                        nc.sync.dma_start(
                            cf[:],
                            gout[a][lr * 128:(lr + 1) * 128,
                                    it * 128:(it + 1) * 128])
                        nc.tensor.matmul(yp[:], cf[:], wo_sb[:, cc, :],
                                         start=(cc == 0), stop=(cc == NHC - 1))
                    y_sb = p3o.tile([128, OSL], F32)
                    if it % 2 == 0:
                        nc.vector.tensor_copy(y_sb[:], yp[:])
                    else:
                        nc.scalar.copy(y_sb[:], yp[:])
                    nc.sync.dma_start(out[it * 128:(it + 1) * 128, :], y_sb[:])

    nc.compile()
    return nc


def _get_nc(mask_mode):
    if mask_mode not in _CACHED:
        _CACHED[mask_mode] = _build(mask_mode)
    return _CACHED[mask_mode]


def _make_in_maps(x, attention_mask, position_ids, Wq, Wk, Wv, Wo, mask_mode):
    x = np.asarray(x, dtype=np.float32)
    attention_mask = np.asarray(attention_mask, dtype=np.float32)
    position_ids = np.asarray(position_ids)
    Wq = np.asarray(Wq, dtype=np.float32)
    Wk = np.asarray(Wk, dtype=np.float32)
    Wv = np.asarray(Wv, dtype=np.float32)
    Wo = np.asarray(Wo, dtype=np.float32)

    in_maps = []
    for c in range(8):
        b, l = c // TP, c % TP
        pos = position_ids[b].astype(np.float32) * 0.01
        mid = 0.5 * (pos.max() + pos.min())
        posq_b = np.ascontiguousarray(
            np.broadcast_to(pos[None, :], (128, S))).astype(np.float32)
        posk_b = np.ascontiguousarray(
            np.broadcast_to((pos - mid)[None, :], (128, S))).astype(np.float32)

        # Wo columns permuted to the gathered order: block a=h, rank lr ->
        # global head 4*lr + h
        wo_sl = Wo[OSL * l:OSL * (l + 1), :]                       # [512, H]
        cols = [wo_sl[:, (4 * lr + h) * HD:(4 * lr + h + 1) * HD]
                for h in range(QH) for lr in range(TP)]
        wo_perm = np.concatenate(cols, axis=1)                     # [512, H]

        maskb_b = (-1e9 * (1.0 - attention_mask[b]))[None, :].astype(np.float32)

        in_maps.append({
            "xt": np.ascontiguousarray(x[b].T),
            "wqt": np.ascontiguousarray(Wq[OSL * l:OSL * (l + 1), :].T),
            "wkt": np.ascontiguousarray(Wk[HD * l:HD * (l + 1), :].T),
            "wvt": np.ascontiguousarray(Wv[HD * l:HD * (l + 1), :].T),
            "wot": np.ascontiguousarray(wo_perm.T),
            "posq": posq_b,
            "posk": posk_b,
            "maskb": np.ascontiguousarray(maskb_b),
        })
    return in_maps


def _run(x, attention_mask, position_ids, Wq, Wk, Wv, Wo, trace=False):
    am = np.asarray(attention_mask, dtype=np.float32)
    if np.all(am == 1.0):
        mask_mode = "ones"
    elif np.all((am == 0.0) | (am == 1.0)):
        mask_mode = "binary"
    else:
        mask_mode = "binary"  # fractional masks unsupported exactly; best effort

    nc = _get_nc(mask_mode)
    in_maps = _make_in_maps(x, attention_mask, position_ids, Wq, Wk, Wv, Wo,
                            mask_mode)
    res = run_bass_kernel_spmd(nc, in_maps, core_ids=list(range(8)),
                               trace=trace)
    y = np.empty((B, S, H), dtype=np.float32)
    for c in range(8):
        b, l = c // TP, c % TP
        y[b][:, OSL * l:OSL * (l + 1)] = res.results[c]["out"]
    return y, res


def kernel(**inputs):
    y, _ = _run(**inputs, trace=False)
    return y


def kernel_profiled(**inputs):
    y, res = _run(**inputs, trace=True)
    return y, res
